# revision 5
# baseline (speedup 1.0000x reference)
"""Multi-head causal attention (b=4, n=2048, d_model=1024, 16 heads) on 8
Trainium2 NeuronCores.

Sharding: core c = (batch b = c//2, head-group hg = c%2); each core computes
one batch with 8 heads (tensor-parallel split of w_q/w_k/w_v by rows and w_o
by columns) and returns a partial [2048, 1024] output; host sums the two
head-group partials per batch.

v2 vs baseline:
- Scores run in fp8e4 DoubleRow mode (0.5 PE cycles/row): q/k projections
  stay fp16 for accuracy, but their outputs are cast straight to fp8 and
  regrouped (flat sbuf->sbuf DMA, [128,512] -> [64,2,512]) into the
  DoubleRow pairing d = 2p + i.
- Scores/exp/PV are trimmed to the causal window on diagonal blocks
  (baseline only trimmed PV).
- The causal mask multiply runs on the idle GpSimd engine against a single
  [128,2,128] triangular window instead of DVE x [128,1024].
- Projection and O-projection PE work is woven into the attention ladder
  as filler units so the PE never idles waiting on exp; ACT (exp) and PE
  stay concurrently busy instead of phase-serialized.
"""

from collections import deque

import numpy as np

B = 4
N = 2048
D_MODEL = 1024
DK = 64
NT = 4          # q tiles of 512
QT = 512        # q tile size
N_CORES = 8

_CACHE = {}


def _split_sync_waits(nc, max_waits=1):
    """walrus on this image allows only 1 sync-wait command per instruction;
    hoist excess waits onto same-engine NoOps inserted just before."""
    import concourse.mybir as mybir

    n_split = 0
    for fn in nc.m.functions:
        for blk in fn.blocks:
            insts = list(blk.instructions)
            out = []
            for inst in insts:
                si = inst.sync_info
                if si is not None and len(si.on_wait) > max_waits:
                    waits = list(si.on_wait)
                    head, rest = waits[:-max_waits], waits[-max_waits:]
                    while head:
                        chunk, head = head[:max_waits], head[max_waits:]
                        nop = mybir.InstNoOp(
                            name=f"{inst.name}-ws{n_split}-{len(out)}",
                            engine=inst.engine,
                            opcode="NoOp",
                            sync_info=mybir.SyncInfo(on_wait=chunk, on_update=[]),
                            bass_nofuse=True,
                        )
                        out.append(nop)
                    si.on_wait = rest
                    n_split += 1
                out.append(inst)
            if len(out) != len(insts):
                blk.instructions = out
    return n_split


def build_nc():
    import concourse.bass as bass
    import concourse.mybir as mybir
    import concourse.tile as tile
    from concourse.bass import ts

    F32 = mybir.dt.float32
    F16 = mybir.dt.float16
    F8 = mybir.dt.float8e4
    AF = mybir.ActivationFunctionType
    DR = mybir.MatmulPerfMode.DoubleRow

    nc = bass.Bass("TRN2", target_bir_lowering=False, debug=False)

    qT_d = nc.dram_tensor("qT", [D_MODEL, N], F16, kind="ExternalInput")
    kT_d = nc.dram_tensor("kT", [D_MODEL, N], F16, kind="ExternalInput")
    vT_d = nc.dram_tensor("vT", [D_MODEL, N], F16, kind="ExternalInput")
    wqT_d = nc.dram_tensor("wqT", [D_MODEL, 512], F16, kind="ExternalInput")
    wkT_d = nc.dram_tensor("wkT", [D_MODEL, 512], F16, kind="ExternalInput")
    wvT_d = nc.dram_tensor("wvT", [D_MODEL, 512], F16, kind="ExternalInput")
    woT_d = nc.dram_tensor("woT", [512, D_MODEL], F16, kind="ExternalInput")
    maskw_d = nc.dram_tensor("maskw", [128, 2, 128], F16, kind="ExternalInput")
    onescol_d = nc.dram_tensor("onescol", [128, 8], F16, kind="ExternalInput")
    sel_d = nc.dram_tensor("sel", [8, 4, 128], F16, kind="ExternalInput")
    sel6_d = nc.dram_tensor("sel6", [6, 3, 128], F16, kind="ExternalInput")
    sel2_d = nc.dram_tensor("sel2", [2, 128], F16, kind="ExternalInput")
    out_d = nc.dram_tensor("out", [N, D_MODEL], F32, kind="ExternalOutput")

    with (
        tile.TileContext(nc) as tc,
        nc.allow_low_precision(reason="fp8/fp16 matmuls are intentional"),
    ):
        with (
            tc.tile_pool(name="persist", bufs=1) as persist,
            tc.tile_pool(name="pt_pool", bufs=1) as pt_pool,
            tc.tile_pool(name="xs", bufs=2) as xs,
            tc.tile_pool(name="outp", bufs=1) as outp,
            tc.tile_pool(name="ps", bufs=1, space="PSUM") as ps,
        ):
            # ---- persistent SBUF tensors ----
            # q/k in fp16, heads packed 2-per-partition-group: head pair g
            # lives at [0:64] (even head) / [64:128] (odd head), plane g
            qT_all = persist.tile([128, 4, N], F16)
            kT_all = persist.tile([128, 4, N], F16)
            v_all = persist.tile([128, 16, 8, 65], F16)  # [key, sb, head, d+1]
            maskw_sb = persist.tile([128, 2, 128], F16)
            onescol_sb = persist.tile([128, 8], F16)
            sel_sb = persist.tile([8, 4, 128], F16)
            sel6_sb = persist.tile([6, 3, 128], F16)
            sel2_sb = persist.tile([2, 128], F16)
            rs3b = persist.tile([2, QT], F32)
            recip3b = persist.tile([2, QT], F16)
            wq_sb = persist.tile([128, 8, 512], F16)
            wk_sb = persist.tile([128, 8, 512], F16)
            wv_sb = persist.tile([128, 8, 512], F16)
            wo_sb = persist.tile([128, 4, D_MODEL], F16)
            ot_sb = [
                persist.tile([128, 4, QT], F16, name=f"ot_sb{t}", tag=f"ot{t}")
                for t in range(NT)
            ]
            rs_sb = [
                persist.tile([8, QT], F32, name=f"rs_sb{t}", tag=f"rs{t}")
                for t in range(NT)
            ]
            recip_sb = [
                persist.tile([8, QT], F16, name=f"recip{t}", tag=f"rc{t}")
                for t in range(NT)
            ]

            # DMA order matters at startup: the v-projection path (wv + vT
            # tile 0) is needed first; wo only at the first O-projection.
            for kc in range(8):
                nc.sync.dma_start(out=wv_sb[:, kc, :], in_=wvT_d[ts(kc, 128), :])
            nc.sync.dma_start(out=maskw_sb, in_=maskw_d[:, :, :])
            nc.sync.dma_start(out=onescol_sb, in_=onescol_d[:, :])
            nc.sync.dma_start(out=sel_sb, in_=sel_d[:, :, :])
            nc.sync.dma_start(out=sel6_sb, in_=sel6_d[:, :, :])
            nc.sync.dma_start(out=sel2_sb, in_=sel2_d[:, :])

            # ---- PE p-state warmup while initial DMAs land ----
            junk = persist.tile([128, 640], F16)
            nc.vector.memset(junk, 0.0)
            pwarm = ps.tile([128, QT], F32, name="pwarm", tag="pj", bufs=2)
            for _ in range(14):
                nc.tensor.matmul(
                    pwarm, junk[:, 0:128], junk[:, 128:640], start=True, stop=True
                )

            # ---------- projection emitters (per q/k/v tile m-block) -------
            def load_x_tiles(t, weights_between=False):
                """DMA the x operand tiles for projection tile t (v first —
                its units run first). At startup the q/k weight DMAs are
                interleaved so each projection's operands arrive together."""
                tsl = ts(t, QT)
                tiles = {}
                for key, src_d in (("v", vT_d), ("k", kT_d), ("q", qT_d)):
                    if weights_between and key == "q":
                        for kc in range(8):
                            nc.sync.dma_start(
                                out=wq_sb[:, kc, :], in_=wqT_d[ts(kc, 128), :]
                            )
                    x = xs.tile(
                        [128, 8, QT], F16, name=f"x{key}", tag=f"x{key}", bufs=2
                    )
                    for kc in range(8):
                        nc.sync.dma_start(out=x[:, kc, :], in_=src_d[ts(kc, 128), tsl])
                    if weights_between and key == "v":
                        for kc in range(8):
                            nc.sync.dma_start(
                                out=wk_sb[:, kc, :], in_=wkT_d[ts(kc, 128), :]
                            )
                    tiles[key] = x
                return tiles

            def proj_qk_unit(which, x, t, m):
                """One m-block of the q or k projection (fp16)."""
                tsl = ts(t, QT)
                w_sb = wq_sb if which == "q" else wk_sb
                dst = qT_all if which == "q" else kT_all
                pj = ps.tile([128, QT], F32, name="pj", tag="pj", bufs=2)
                for kc in range(8):
                    nc.tensor.matmul(
                        pj,
                        w_sb[:, kc, ts(m, 128)],
                        x[:, kc, :],
                        start=(kc == 0),
                        stop=(kc == 7),
                    )
                nc.vector.tensor_copy(dst[:, m, tsl], pj)

            def proj_v_unit(x, t, m):
                """One m-block of the v projection (natural [seq, d] layout
                plus the ones column feeding softmax denominators)."""
                pj = ps.tile([128, QT], F32, name="pj", tag="pj", bufs=2)
                for kc in range(8):
                    nc.tensor.matmul(
                        pj,
                        x[:, kc, ts(m, 128)],
                        wv_sb[:, kc, :],
                        start=(kc == 0),
                        stop=(kc == 7),
                    )
                sb = t * 4 + m
                nc.vector.tensor_copy(
                    v_all[:, sb, :, 0:64],
                    pj[:, :].rearrange("p (h d) -> p h d", h=8),
                )
                nc.vector.tensor_copy(v_all[:, sb, :, 64], onescol_sb)

            def proj_tile_units(t, x=None):
                """Filler units projecting tile t (v first: attention tile t
                consumes v_all[4t..4t+3] earliest via PV j=4t)."""
                if x is None:
                    x = load_x_tiles(t)
                units = []
                for m in range(4):
                    units.append(lambda m=m, x=x["v"]: proj_v_unit(x, t, m))
                for m in range(4):
                    units.append(lambda m=m, x=x["k"]: proj_qk_unit("k", x, t, m))
                for m in range(4):
                    units.append(lambda m=m, x=x["q"]: proj_qk_unit("q", x, t, m))
                return units

            # ---------- normalize + O-projection emitters ------------------
            def norm_unit(t):
                last = t == NT - 1
                if last:
                    # rows 0:6 (head pairs g=0..2) were reciprocal'd early,
                    # inside the ladder; only g=3's rows remain on the tail
                    nc.vector.reciprocal(recip3b, rs3b)
                else:
                    nc.vector.reciprocal(recip_sb[t], rs_sb[t])
                for g in range(4):
                    bc = ps.tile([128, QT], F32, name="bc", tag="pj", bufs=2)
                    if last and g == 3:
                        nc.tensor.matmul(
                            bc, sel2_sb[:, :], recip3b[:, :],
                            start=True, stop=True,
                        )
                    elif last:
                        nc.tensor.matmul(
                            bc, sel6_sb[:, g, :], recip_sb[t][0:6, :],
                            start=True, stop=True,
                        )
                    else:
                        nc.tensor.matmul(
                            bc, sel_sb[:, g, :], recip_sb[t][:, :],
                            start=True, stop=True,
                        )
                    nc.vector.tensor_mul(ot_sb[t][:, g, :], ot_sb[t][:, g, :], bc)

            def oproj_unit(t, mm, n2):
                po = ps.tile([128, 512], F32, name="po", tag="pj", bufs=2)
                for g in range(4):
                    nc.tensor.matmul(
                        po,
                        ot_sb[t][:, g, ts(mm, 128)],
                        wo_sb[:, g, ts(n2, 512)],
                        start=(g == 0),
                        stop=(g == 3),
                    )
                ob = outp.tile([128, 512], F32, name="ob", tag="ob", bufs=3)
                nc.vector.tensor_copy(ob, po)
                nc.sync.dma_start(
                    out=out_d[ts(4 * t + mm, 128), ts(n2, 512)], in_=ob
                )

            def oproj_tile_units(t):
                units = [lambda: norm_unit(t)]
                for mm in range(4):
                    for n2 in range(2):
                        units.append(
                            lambda mm=mm, n2=n2: oproj_unit(t, mm, n2)
                        )
                return units

            # ================= main schedule =================
            # proj(t0) up front; proj(t+1) and oproj(t-1) woven into the
            # attention ladder as PE filler between blocks.
            filler = deque()
            # tile-0: run v fully plus k/q m=0 up front; m>=1 k/q units are
            # woven into the tile-0 ladder (attention head pair g only needs
            # m-block g of qT_all/kT_all)
            x0 = load_x_tiles(0, weights_between=True)
            for g in range(4):
                nc.sync.dma_start(out=wo_sb[:, g, :], in_=woT_d[ts(g, 128), :])
            units0 = proj_tile_units(0, x0)
            for u in units0[:4] + [units0[4], units0[8]]:  # v0..3, k0, q0
                u()
            emitted0 = {1: 0, 2: 0, 3: 0}

            def wrap0(m, u):
                def f(m=m, u=u):
                    u()
                    emitted0[m] += 1
                return f

            for t in range(NT):
                nkb = 4 * t + 4  # causal: key blocks 0 .. 4t+3
                if t == 0:
                    # m>=1 k/q units of tile 0, ahead of tile-1 units; the
                    # g-boundary check below guarantees emission in time
                    for m in (1, 2, 3):
                        filler.append(wrap0(m, units0[4 + m]))
                        filler.append(wrap0(m, units0[8 + m]))
                if t + 1 < NT:
                    filler.extend(proj_tile_units(t + 1))
                if t > 0:
                    filler.extend(oproj_tile_units(t - 1))
                blocks_left = 4 * nkb
                stride = max(1, (4 * nkb) // max(1, len(filler)))
                since = 0
                for g in range(4):
                    if t == 0 and g >= 1:
                        # head pair g's scores need tile-0 m-block g projected
                        while filler and emitted0[g] < 2:
                            filler.popleft()()
                    ota = ps.tile([65, QT], F32, name="ota", tag="ota", bufs=1)
                    otb = ps.tile([65, QT], F32, name="otb", tag="otb", bufs=1)

                    sp_live = {}

                    def emit_scores(j, t=t, g=g, sp_live=sp_live):
                        r = j - 4 * t
                        z = 128 * r if r > 0 else 0
                        sp = ps.tile(
                            [128, 2, QT], F32, name="sp", tag="sp", bufs=2
                        )
                        for b2 in range(2):
                            pb = 64 * b2
                            nc.tensor.matmul(
                                sp[:, b2, z:QT],
                                kT_all[pb : pb + 64, g, ts(j, 128)],
                                qT_all[pb : pb + 64, g, t * QT + z : (t + 1) * QT],
                                start=True,
                                stop=True,
                                tile_position=(pb, 0),
                            )
                        sp_live[j] = (sp, z)

                    # scores run one block ahead of exp/PV so the PE never
                    # sits on the exp dependency
                    emit_scores(0)
                    for j in range(nkb):
                        if j + 1 < nkb:
                            emit_scores(j + 1)
                        sp, z = sp_live.pop(j)
                        r = j - 4 * t
                        pt2 = pt_pool.tile(
                            [128, 2, QT], F16, name="pt2", tag="pt2", bufs=6
                        )
                        nc.scalar.activation(
                            pt2[:, :, z:QT], sp[:, :, z:QT], AF.Exp, scale=0.125
                        )
                        if r >= 0:
                            nc.gpsimd.tensor_mul(
                                pt2[:, :, z : z + 128],
                                pt2[:, :, z : z + 128],
                                maskw_sb,
                            )
                        nc.tensor.matmul(
                            ota[:, z:QT],
                            v_all[:, j, 2 * g, :],
                            pt2[:, 0, z:QT],
                            start=(j == 0),
                            stop=(j == nkb - 1),
                        )
                        nc.tensor.matmul(
                            otb[:, z:QT],
                            v_all[:, j, 2 * g + 1, :],
                            pt2[:, 1, z:QT],
                            start=(j == 0),
                            stop=(j == nkb - 1),
                        )
                        # weave pending proj/oproj PE work between blocks,
                        # spread evenly and guaranteed drained by tile end
                        since += 1
                        if filler and since >= stride:
                            filler.popleft()()
                            since = 0
                        while filler and len(filler) >= blocks_left:
                            filler.popleft()()
                        blocks_left -= 1
                    # stage O^T and rowsums to SBUF
                    nc.vector.tensor_copy(ot_sb[t][0:64, g, :], ota[0:64, :])
                    nc.vector.tensor_copy(ot_sb[t][64:128, g, :], otb[0:64, :])
                    tmp_rs = pt_pool.tile(
                        [1, 2, QT], F32, name="tmp_rs", tag="tmp_rs", bufs=2
                    )
                    nc.vector.tensor_copy(tmp_rs[0:1, 0, :], ota[64:65, :])
                    nc.vector.tensor_copy(tmp_rs[0:1, 1, :], otb[64:65, :])
                    if t == NT - 1 and g == 3:
                        nc.sync.dma_start(out=rs3b[:, :], in_=tmp_rs[0:1, :, :])
                    else:
                        nc.sync.dma_start(
                            out=rs_sb[t][2 * g : 2 * g + 2, :],
                            in_=tmp_rs[0:1, :, :],
                        )
                    if t == NT - 1 and g == 2:
                        # take g=0..2's reciprocals off the final-tile tail
                        nc.vector.reciprocal(
                            recip_sb[t][0:6, :], rs_sb[t][0:6, :]
                        )
            for u in filler:
                u()
            for u in oproj_tile_units(NT - 1):
                u()

    _split_sync_waits(nc)
    return nc


def _prep_inputs(Q, K, V, w_q, w_k, w_v, w_o):
    """Build the 8 per-core input maps (host-side shard + transpose + cast)."""
    Q = np.asarray(Q, dtype=np.float32)
    K = np.asarray(K, dtype=np.float32)
    V = np.asarray(V, dtype=np.float32)
    w_q = np.asarray(w_q, dtype=np.float32)
    w_k = np.asarray(w_k, dtype=np.float32)
    w_v = np.asarray(w_v, dtype=np.float32)
    w_o = np.asarray(w_o, dtype=np.float32)

    k_idx = np.arange(128)[:, None]
    w_idx = np.arange(128)[None, :]
    maskw = np.zeros((128, 2, 128), dtype=np.float16)
    maskw[:, 0, :] = (k_idx <= w_idx).astype(np.float16)
    maskw[:, 1, :] = maskw[:, 0, :]
    onescol = np.ones((128, 8), dtype=np.float16)
    sel = np.zeros((8, 4, 128), dtype=np.float16)
    for g in range(4):
        sel[2 * g, g, 0:64] = 1.0
        sel[2 * g + 1, g, 64:128] = 1.0
    sel6 = np.zeros((6, 3, 128), dtype=np.float16)
    for g in range(3):
        sel6[2 * g, g, 0:64] = 1.0
        sel6[2 * g + 1, g, 64:128] = 1.0
    sel2 = np.zeros((2, 128), dtype=np.float16)
    sel2[0, 0:64] = 1.0
    sel2[1, 64:128] = 1.0

    def f16T(a):
        return np.ascontiguousarray(a.T).astype(np.float16)

    qT = [f16T(Q[b]) for b in range(B)]
    kT = [f16T(K[b]) for b in range(B)]
    vT = [f16T(V[b]) for b in range(B)]
    wqT = [f16T(w_q[hg * 512 : hg * 512 + 512, :]) for hg in range(2)]
    wkT = [f16T(w_k[hg * 512 : hg * 512 + 512, :]) for hg in range(2)]
    wvT = [f16T(w_v[hg * 512 : hg * 512 + 512, :]) for hg in range(2)]
    woT = [f16T(w_o[:, hg * 512 : hg * 512 + 512]) for hg in range(2)]

    in_maps = []
    for c in range(N_CORES):
        b, hg = c // 2, c % 2
        in_maps.append(
            {
                "qT": qT[b],
                "kT": kT[b],
                "vT": vT[b],
                "wqT": wqT[hg],
                "wkT": wkT[hg],
                "wvT": wvT[hg],
                "woT": woT[hg],
                "maskw": maskw,
                "onescol": onescol,
                "sel": sel,
                "sel6": sel6,
                "sel2": sel2,
            }
        )
    return in_maps


def kernel(Q, K, V, w_q, w_k, w_v, w_o, _trace=False):
    from concourse.bass_utils import run_bass_kernel_spmd

    if "nc" not in _CACHE:
        _CACHE["nc"] = build_nc()
    nc = _CACHE["nc"]

    in_maps = _prep_inputs(Q, K, V, w_q, w_k, w_v, w_o)
    res = run_bass_kernel_spmd(
        nc, in_maps, core_ids=list(range(N_CORES)), trace=_trace
    )
    outs = [r["out"] for r in res.results]
    full = np.empty((B, N, D_MODEL), dtype=np.float32)
    for b in range(B):
        full[b] = outs[2 * b] + outs[2 * b + 1]
    if _trace:
        _CACHE["last_result"] = res
    return full


# revision 6
# speedup vs baseline: 1.0211x; 1.0211x over previous
"""Multi-head causal attention (b=4, n=2048, d_model=1024, 16 heads) on 8
Trainium2 NeuronCores.

Sharding: core c = (batch b = c//2, head-group hg = c%2); each core computes
one batch with 8 heads (tensor-parallel split of w_q/w_k/w_v by rows and w_o
by columns) and returns a partial [2048, 1024] output; host sums the two
head-group partials per batch.

All-fp16 datapath (fp8 DoubleRow measured no faster on this hardware —
matmul wall time tracks output columns, not rows); the wins over the
baseline are scheduling and work-trimming:
- Scores/exp/PV are all trimmed to the causal window on diagonal blocks
  (baseline only trimmed PV), and the causal mask multiply runs on the
  idle GpSimd engine against one [128,2,128] triangular window instead
  of DVE x [128,1024].
- Projection and O-projection PE work is woven into the attention ladder
  as filler units, scores are software-pipelined one key-block ahead of
  exp/PV, and phase transitions (startup DMA order, tile-0 deferred m>=1
  q/k units, final-tile reciprocal split) are overlapped, so the PE stays
  near its fp16 column-throughput roofline for the whole kernel.
"""

from collections import deque

import numpy as np

B = 4
N = 2048
D_MODEL = 1024
DK = 64
NT = 4          # q tiles of 512
QT = 512        # q tile size
N_CORES = 8

_CACHE = {}


def _split_sync_waits(nc, max_waits=1):
    """walrus on this image allows only 1 sync-wait command per instruction;
    hoist excess waits onto same-engine NoOps inserted just before."""
    import concourse.mybir as mybir

    n_split = 0
    for fn in nc.m.functions:
        for blk in fn.blocks:
            insts = list(blk.instructions)
            out = []
            for inst in insts:
                si = inst.sync_info
                if si is not None and len(si.on_wait) > max_waits:
                    waits = list(si.on_wait)
                    head, rest = waits[:-max_waits], waits[-max_waits:]
                    while head:
                        chunk, head = head[:max_waits], head[max_waits:]
                        nop = mybir.InstNoOp(
                            name=f"{inst.name}-ws{n_split}-{len(out)}",
                            engine=inst.engine,
                            opcode="NoOp",
                            sync_info=mybir.SyncInfo(on_wait=chunk, on_update=[]),
                            bass_nofuse=True,
                        )
                        out.append(nop)
                    si.on_wait = rest
                    n_split += 1
                out.append(inst)
            if len(out) != len(insts):
                blk.instructions = out
    return n_split


def build_nc():
    import concourse.bass as bass
    import concourse.mybir as mybir
    import concourse.tile as tile
    from concourse.bass import ts

    F32 = mybir.dt.float32
    F16 = mybir.dt.float16
    F8 = mybir.dt.float8e4
    AF = mybir.ActivationFunctionType
    DR = mybir.MatmulPerfMode.DoubleRow

    nc = bass.Bass("TRN2", target_bir_lowering=False, debug=False)

    qT_d = nc.dram_tensor("qT", [D_MODEL, N], F16, kind="ExternalInput")
    kT_d = nc.dram_tensor("kT", [D_MODEL, N], F16, kind="ExternalInput")
    vT_d = nc.dram_tensor("vT", [D_MODEL, N], F16, kind="ExternalInput")
    wqT_d = nc.dram_tensor("wqT", [D_MODEL, 512], F16, kind="ExternalInput")
    wkT_d = nc.dram_tensor("wkT", [D_MODEL, 512], F16, kind="ExternalInput")
    wvT_d = nc.dram_tensor("wvT", [D_MODEL, 512], F16, kind="ExternalInput")
    woT_d = nc.dram_tensor("woT", [512, D_MODEL], F16, kind="ExternalInput")
    maskw_d = nc.dram_tensor("maskw", [128, 2, 128], F16, kind="ExternalInput")
    onescol_d = nc.dram_tensor("onescol", [128, 8], F16, kind="ExternalInput")
    sel_d = nc.dram_tensor("sel", [8, 4, 128], F16, kind="ExternalInput")
    sel6_d = nc.dram_tensor("sel6", [6, 3, 128], F16, kind="ExternalInput")
    sel2_d = nc.dram_tensor("sel2", [2, 128], F16, kind="ExternalInput")
    out_d = nc.dram_tensor("out", [N, D_MODEL], F32, kind="ExternalOutput")

    with (
        tile.TileContext(nc) as tc,
        nc.allow_low_precision(reason="fp8/fp16 matmuls are intentional"),
    ):
        with (
            tc.tile_pool(name="persist", bufs=1) as persist,
            tc.tile_pool(name="pt_pool", bufs=1) as pt_pool,
            tc.tile_pool(name="xs", bufs=2) as xs,
            tc.tile_pool(name="outp", bufs=1) as outp,
            tc.tile_pool(name="ps", bufs=1, space="PSUM") as ps,
        ):
            # ---- persistent SBUF tensors ----
            # q/k in fp16, heads packed 2-per-partition-group: head pair g
            # lives at [0:64] (even head) / [64:128] (odd head), plane g
            qT_all = persist.tile([128, 4, N], F16)
            kT_all = persist.tile([128, 4, N], F16)
            v_all = persist.tile([128, 16, 8, 65], F16)  # [key, sb, head, d+1]
            maskw_sb = persist.tile([128, 2, 128], F16)
            onescol_sb = persist.tile([128, 8], F16)
            sel_sb = persist.tile([8, 4, 128], F16)
            sel6_sb = persist.tile([6, 3, 128], F16)
            sel2_sb = persist.tile([2, 128], F16)
            rs3b = persist.tile([2, QT], F32)
            recip3b = persist.tile([2, QT], F16)
            wq_sb = persist.tile([128, 8, 512], F16)
            wk_sb = persist.tile([128, 8, 512], F16)
            wv_sb = persist.tile([128, 8, 512], F16)
            wo_sb = persist.tile([128, 4, D_MODEL], F16)
            ot_sb = [
                persist.tile([128, 4, QT], F16, name=f"ot_sb{t}", tag=f"ot{t}")
                for t in range(NT)
            ]
            rs_sb = [
                persist.tile([8, QT], F32, name=f"rs_sb{t}", tag=f"rs{t}")
                for t in range(NT)
            ]
            recip_sb = [
                persist.tile([8, QT], F16, name=f"recip{t}", tag=f"rc{t}")
                for t in range(NT)
            ]

            # DMA order matters at startup: the v-projection path (wv + vT
            # tile 0) is needed first; wo only at the first O-projection.
            for kc in range(8):
                nc.sync.dma_start(out=wv_sb[:, kc, :], in_=wvT_d[ts(kc, 128), :])
            nc.sync.dma_start(out=maskw_sb, in_=maskw_d[:, :, :])
            nc.sync.dma_start(out=onescol_sb, in_=onescol_d[:, :])
            nc.sync.dma_start(out=sel_sb, in_=sel_d[:, :, :])
            nc.sync.dma_start(out=sel6_sb, in_=sel6_d[:, :, :])
            nc.sync.dma_start(out=sel2_sb, in_=sel2_d[:, :])

            # ---- PE p-state warmup while initial DMAs land ----
            junk = persist.tile([128, 640], F16)
            nc.vector.memset(junk, 0.0)
            pwarm = ps.tile([128, QT], F32, name="pwarm", tag="pj", bufs=2)
            for _ in range(24):
                nc.tensor.matmul(
                    pwarm, junk[:, 0:128], junk[:, 128:640], start=True, stop=True
                )

            # ---------- projection emitters (per q/k/v tile m-block) -------
            def load_x_tiles(t, weights_between=False):
                """DMA the x operand tiles for projection tile t (v first —
                its units run first). At startup the q/k weight DMAs are
                interleaved so each projection's operands arrive together."""
                tsl = ts(t, QT)
                tiles = {}
                for key, src_d in (("v", vT_d), ("k", kT_d), ("q", qT_d)):
                    if weights_between and key == "q":
                        for kc in range(8):
                            nc.sync.dma_start(
                                out=wq_sb[:, kc, :], in_=wqT_d[ts(kc, 128), :]
                            )
                    x = xs.tile(
                        [128, 8, QT], F16, name=f"x{key}", tag=f"x{key}", bufs=2
                    )
                    for kc in range(8):
                        nc.sync.dma_start(out=x[:, kc, :], in_=src_d[ts(kc, 128), tsl])
                    if weights_between and key == "v":
                        for kc in range(8):
                            nc.sync.dma_start(
                                out=wk_sb[:, kc, :], in_=wkT_d[ts(kc, 128), :]
                            )
                    tiles[key] = x
                return tiles

            def proj_qk_unit(which, x, t, m):
                """One m-block of the q or k projection (fp16)."""
                tsl = ts(t, QT)
                w_sb = wq_sb if which == "q" else wk_sb
                dst = qT_all if which == "q" else kT_all
                pj = ps.tile([128, QT], F32, name="pj", tag="pj", bufs=2)
                for kc in range(8):
                    nc.tensor.matmul(
                        pj,
                        w_sb[:, kc, ts(m, 128)],
                        x[:, kc, :],
                        start=(kc == 0),
                        stop=(kc == 7),
                    )
                nc.vector.tensor_copy(dst[:, m, tsl], pj)

            def proj_v_unit(x, t, m):
                """One m-block of the v projection (natural [seq, d] layout
                plus the ones column feeding softmax denominators)."""
                pj = ps.tile([128, QT], F32, name="pj", tag="pj", bufs=2)
                for kc in range(8):
                    nc.tensor.matmul(
                        pj,
                        x[:, kc, ts(m, 128)],
                        wv_sb[:, kc, :],
                        start=(kc == 0),
                        stop=(kc == 7),
                    )
                sb = t * 4 + m
                nc.vector.tensor_copy(
                    v_all[:, sb, :, 0:64],
                    pj[:, :].rearrange("p (h d) -> p h d", h=8),
                )
                nc.vector.tensor_copy(v_all[:, sb, :, 64], onescol_sb)

            def proj_tile_units(t, x=None):
                """Filler units projecting tile t (v first: attention tile t
                consumes v_all[4t..4t+3] earliest via PV j=4t)."""
                if x is None:
                    x = load_x_tiles(t)
                units = []
                for m in range(4):
                    units.append(lambda m=m, x=x["v"]: proj_v_unit(x, t, m))
                for m in range(4):
                    units.append(lambda m=m, x=x["k"]: proj_qk_unit("k", x, t, m))
                for m in range(4):
                    units.append(lambda m=m, x=x["q"]: proj_qk_unit("q", x, t, m))
                return units

            # ---------- normalize + O-projection emitters ------------------
            def norm_unit(t):
                if t == NT - 1:
                    # head pairs g=0..2 were normalized inside the ladder;
                    # only g=3's reciprocal/broadcast/rescale remain here
                    nc.vector.reciprocal(recip3b, rs3b)
                    bc = ps.tile([128, QT], F32, name="bc", tag="pj", bufs=2)
                    nc.tensor.matmul(
                        bc, sel2_sb[:, :], recip3b[:, :], start=True, stop=True
                    )
                    nc.vector.tensor_mul(
                        ot_sb[t][:, 3, :], ot_sb[t][:, 3, :], bc
                    )
                    return
                nc.vector.reciprocal(recip_sb[t], rs_sb[t])
                for g in range(4):
                    bc = ps.tile([128, QT], F32, name="bc", tag="pj", bufs=2)
                    nc.tensor.matmul(
                        bc, sel_sb[:, g, :], recip_sb[t][:, :],
                        start=True, stop=True,
                    )
                    nc.vector.tensor_mul(ot_sb[t][:, g, :], ot_sb[t][:, g, :], bc)

            def oproj_unit(t, mm, n2):
                po = ps.tile([128, 512], F32, name="po", tag="pj", bufs=2)
                for g in range(4):
                    nc.tensor.matmul(
                        po,
                        ot_sb[t][:, g, ts(mm, 128)],
                        wo_sb[:, g, ts(n2, 512)],
                        start=(g == 0),
                        stop=(g == 3),
                    )
                ob = outp.tile([128, 512], F32, name="ob", tag="ob", bufs=3)
                nc.vector.tensor_copy(ob, po)
                nc.sync.dma_start(
                    out=out_d[ts(4 * t + mm, 128), ts(n2, 512)], in_=ob
                )

            def oproj_tile_units(t):
                units = [lambda: norm_unit(t)]
                for mm in range(4):
                    for n2 in range(2):
                        units.append(
                            lambda mm=mm, n2=n2: oproj_unit(t, mm, n2)
                        )
                return units

            # ================= main schedule =================
            # proj(t0) up front; proj(t+1) and oproj(t-1) woven into the
            # attention ladder as PE filler between blocks.
            filler = deque()
            # tile-0: run v fully plus k/q m=0 up front; m>=1 k/q units are
            # woven into the tile-0 ladder (attention head pair g only needs
            # m-block g of qT_all/kT_all)
            x0 = load_x_tiles(0, weights_between=True)
            for g in range(4):
                nc.sync.dma_start(out=wo_sb[:, g, :], in_=woT_d[ts(g, 128), :])
            units0 = proj_tile_units(0, x0)
            for u in units0[:4] + [units0[4], units0[8]]:  # v0..3, k0, q0
                u()
            emitted0 = {1: 0, 2: 0, 3: 0}

            def wrap0(m, u):
                def f(m=m, u=u):
                    u()
                    emitted0[m] += 1
                return f

            for t in range(NT):
                nkb = 4 * t + 4  # causal: key blocks 0 .. 4t+3
                if t == 0:
                    # m>=1 k/q units of tile 0, ahead of tile-1 units; the
                    # g-boundary check below guarantees emission in time
                    for m in (1, 2, 3):
                        filler.append(wrap0(m, units0[4 + m]))
                        filler.append(wrap0(m, units0[8 + m]))
                if t + 1 < NT:
                    filler.extend(proj_tile_units(t + 1))
                if t > 0:
                    filler.extend(oproj_tile_units(t - 1))
                blocks_left = 4 * nkb
                stride = max(1, (4 * nkb) // max(1, len(filler)))
                since = 0
                for g in range(4):
                    if t == 0 and g >= 1:
                        # head pair g's scores need tile-0 m-block g projected
                        while filler and emitted0[g] < 2:
                            filler.popleft()()
                    ota = ps.tile([65, QT], F32, name="ota", tag="ota", bufs=1)
                    otb = ps.tile([65, QT], F32, name="otb", tag="otb", bufs=1)

                    sp_live = {}

                    def emit_scores(j, t=t, g=g, sp_live=sp_live):
                        r = j - 4 * t
                        z = 128 * r if r > 0 else 0
                        sp = ps.tile(
                            [128, 2, QT], F32, name="sp", tag="sp", bufs=2
                        )
                        for b2 in range(2):
                            pb = 64 * b2
                            nc.tensor.matmul(
                                sp[:, b2, z:QT],
                                kT_all[pb : pb + 64, g, ts(j, 128)],
                                qT_all[pb : pb + 64, g, t * QT + z : (t + 1) * QT],
                                start=True,
                                stop=True,
                                tile_position=(pb, 0),
                            )
                        sp_live[j] = (sp, z)

                    # scores run one block ahead of exp/PV so the PE never
                    # sits on the exp dependency
                    emit_scores(0)
                    for j in range(nkb):
                        if j + 1 < nkb:
                            emit_scores(j + 1)
                        sp, z = sp_live.pop(j)
                        r = j - 4 * t
                        pt2 = pt_pool.tile(
                            [128, 2, QT], F16, name="pt2", tag="pt2", bufs=6
                        )
                        nc.scalar.activation(
                            pt2[:, :, z:QT], sp[:, :, z:QT], AF.Exp, scale=0.125
                        )
                        if r >= 0:
                            nc.gpsimd.tensor_mul(
                                pt2[:, :, z : z + 128],
                                pt2[:, :, z : z + 128],
                                maskw_sb,
                            )
                        nc.tensor.matmul(
                            ota[:, z:QT],
                            v_all[:, j, 2 * g, :],
                            pt2[:, 0, z:QT],
                            start=(j == 0),
                            stop=(j == nkb - 1),
                        )
                        nc.tensor.matmul(
                            otb[:, z:QT],
                            v_all[:, j, 2 * g + 1, :],
                            pt2[:, 1, z:QT],
                            start=(j == 0),
                            stop=(j == nkb - 1),
                        )
                        # weave pending proj/oproj PE work between blocks,
                        # spread evenly and guaranteed drained by tile end
                        since += 1
                        if filler and since >= stride:
                            filler.popleft()()
                            since = 0
                        while filler and len(filler) >= blocks_left:
                            filler.popleft()()
                        blocks_left -= 1
                    # stage rowsums first (they gate the reciprocal chain),
                    # then O^T, to SBUF
                    tmp_rs = pt_pool.tile(
                        [1, 2, QT], F32, name="tmp_rs", tag="tmp_rs", bufs=2
                    )
                    nc.vector.tensor_copy(tmp_rs[0:1, 0, :], ota[64:65, :])
                    nc.vector.tensor_copy(tmp_rs[0:1, 1, :], otb[64:65, :])
                    if t == NT - 1 and g == 3:
                        nc.sync.dma_start(out=rs3b[:, :], in_=tmp_rs[0:1, :, :])
                    else:
                        nc.sync.dma_start(
                            out=rs_sb[t][2 * g : 2 * g + 2, :],
                            in_=tmp_rs[0:1, :, :],
                        )
                    nc.vector.tensor_copy(ot_sb[t][0:64, g, :], ota[0:64, :])
                    nc.vector.tensor_copy(ot_sb[t][64:128, g, :], otb[0:64, :])
                    if t == NT - 1 and g == 2:
                        # take g=0..2's normalize work off the final-tile
                        # tail: their reciprocals, broadcasts and rescales
                        # only need rows 0:6 of the rowsums
                        nc.vector.reciprocal(
                            recip_sb[t][0:6, :], rs_sb[t][0:6, :]
                        )
                        for gg in range(3):
                            bce = ps.tile(
                                [128, QT], F32, name="bce", tag="pj", bufs=2
                            )
                            nc.tensor.matmul(
                                bce, sel6_sb[:, gg, :], recip_sb[t][0:6, :],
                                start=True, stop=True,
                            )
                            nc.vector.tensor_mul(
                                ot_sb[t][:, gg, :], ot_sb[t][:, gg, :], bce
                            )
            for u in filler:
                u()
            for u in oproj_tile_units(NT - 1):
                u()

    _split_sync_waits(nc)
    return nc


def _prep_inputs(Q, K, V, w_q, w_k, w_v, w_o):
    """Build the 8 per-core input maps (host-side shard + transpose + cast)."""
    Q = np.asarray(Q, dtype=np.float32)
    K = np.asarray(K, dtype=np.float32)
    V = np.asarray(V, dtype=np.float32)
    w_q = np.asarray(w_q, dtype=np.float32)
    w_k = np.asarray(w_k, dtype=np.float32)
    w_v = np.asarray(w_v, dtype=np.float32)
    w_o = np.asarray(w_o, dtype=np.float32)

    k_idx = np.arange(128)[:, None]
    w_idx = np.arange(128)[None, :]
    maskw = np.zeros((128, 2, 128), dtype=np.float16)
    maskw[:, 0, :] = (k_idx <= w_idx).astype(np.float16)
    maskw[:, 1, :] = maskw[:, 0, :]
    onescol = np.ones((128, 8), dtype=np.float16)
    sel = np.zeros((8, 4, 128), dtype=np.float16)
    for g in range(4):
        sel[2 * g, g, 0:64] = 1.0
        sel[2 * g + 1, g, 64:128] = 1.0
    sel6 = np.zeros((6, 3, 128), dtype=np.float16)
    for g in range(3):
        sel6[2 * g, g, 0:64] = 1.0
        sel6[2 * g + 1, g, 64:128] = 1.0
    sel2 = np.zeros((2, 128), dtype=np.float16)
    sel2[0, 0:64] = 1.0
    sel2[1, 64:128] = 1.0

    def f16T(a):
        return np.ascontiguousarray(a.T).astype(np.float16)

    qT = [f16T(Q[b]) for b in range(B)]
    kT = [f16T(K[b]) for b in range(B)]
    vT = [f16T(V[b]) for b in range(B)]
    wqT = [f16T(w_q[hg * 512 : hg * 512 + 512, :]) for hg in range(2)]
    wkT = [f16T(w_k[hg * 512 : hg * 512 + 512, :]) for hg in range(2)]
    wvT = [f16T(w_v[hg * 512 : hg * 512 + 512, :]) for hg in range(2)]
    woT = [f16T(w_o[:, hg * 512 : hg * 512 + 512]) for hg in range(2)]

    in_maps = []
    for c in range(N_CORES):
        b, hg = c // 2, c % 2
        in_maps.append(
            {
                "qT": qT[b],
                "kT": kT[b],
                "vT": vT[b],
                "wqT": wqT[hg],
                "wkT": wkT[hg],
                "wvT": wvT[hg],
                "woT": woT[hg],
                "maskw": maskw,
                "onescol": onescol,
                "sel": sel,
                "sel6": sel6,
                "sel2": sel2,
            }
        )
    return in_maps


def kernel(Q, K, V, w_q, w_k, w_v, w_o, _trace=False):
    from concourse.bass_utils import run_bass_kernel_spmd

    if "nc" not in _CACHE:
        _CACHE["nc"] = build_nc()
    nc = _CACHE["nc"]

    in_maps = _prep_inputs(Q, K, V, w_q, w_k, w_v, w_o)
    res = run_bass_kernel_spmd(
        nc, in_maps, core_ids=list(range(N_CORES)), trace=_trace
    )
    outs = [r["out"] for r in res.results]
    full = np.empty((B, N, D_MODEL), dtype=np.float32)
    for b in range(B):
        full[b] = outs[2 * b] + outs[2 * b + 1]
    if _trace:
        _CACHE["last_result"] = res
    return full
